# revision 7
# baseline (speedup 1.0000x reference)
"""PatternMemory kernel for 8 Trainium2 NeuronCores.

Math (B=8, T=1024, C=1024, P=100):
  ctx_h = context @ W1[:C]                   (B, C)
  trg_h = triggers @ W1[C:]                  (P, C)
  h = relu(ctx_h[:,None,:] + trg_h[None,:,:] + b1)
  logits = h @ W2 + b2[0]                    (B, P)
  scores = sigmoid(logits).mean(axis=0)      (P,)
  w = where(scores > 0.5, scores * conf, 0)
  out = attention_scores + 0.1 * einsum("p,pij->ij", w, biases)

Sharding: core r owns rows [128r, 128(r+1)) of the (T, T) plane. The
(P, T, T) biases tensor — the only big input — splits cleanly along
rows, so every core does the full (cheap) MLP redundantly and there
are no collectives. Per-core DMA: 50MB bias shard + 8MB W1 + 4MB attn
+ 4MB out ~= 66MB -> ~171us at 385GB/s.

MLP formulation: small tensors (trigT blocks [128,100], ctxT blocks
[128,8]) are the matmul stationary, W1 streams through as moving rhs
in [128,512] chunks (16 logical matmuls per half instead of 64 with
W1 stationary — fp32 stationary loads were the old bottleneck). The
[P,C]/[B,C] results are PE-transposed (exact, identity matmul) back
to [c,*] layout for the per-b relu-bias trick and the W2 contraction.
b1 is folded into the ctx matmul via a rank-1 ones row. Bias tiles
stream on the GpSimd DMA queue from t=0; attn loads issue late from
the Scalar queue; final accumulation uses 4 chains merged pairwise.
"""

import numpy as np
import bass_rust

from concourse import bass, mybir
from concourse.bass_utils import run_bass_kernel_spmd
from concourse.tile import TileContext

B, T, C, P = 8, 1024, 1024, 100
NCORES = 8
ROWS = T // NCORES  # 128 rows of the (T, T) plane per core
FP32 = mybir.dt.float32
AF = mybir.ActivationFunctionType
ALU = mybir.AluOpType

SIM_THRESHOLD = 0.5
LAMBDA = 0.1

BIAS_BUFS = 12
CHAIN = 25  # patterns per accumulation chain (4 chains)

_NC_CACHE = {}


def _build_nc() -> bass.Bass:
    nc = bass.Bass("TRN2", target_bir_lowering=False, debug=False,
                   num_devices=NCORES)

    bias_s = nc.dram_tensor("bias_s", (P, ROWS, T), FP32, kind="ExternalInput").ap()
    attn_s = nc.dram_tensor("attn_s", (B, ROWS, T), FP32, kind="ExternalInput").ap()
    # packed W1 halves: [r, ch*4096 + kt*512 + c'] = W1[off + kt*128 + r, ch*512 + c']
    w1hi = nc.dram_tensor("w1hi", (128, 8192), FP32, kind="ExternalInput").ap()
    w1lo = nc.dram_tensor("w1lo", (128, 8192), FP32, kind="ExternalInput").ap()
    trigp = nc.dram_tensor("trigp", (128, 8 * P), FP32, kind="ExternalInput").ap()
    ctxp = nc.dram_tensor("ctxp", (128, 8 * B), FP32, kind="ExternalInput").ap()
    b1row = nc.dram_tensor("b1row", (1, C), FP32, kind="ExternalInput").ap()
    w2r = nc.dram_tensor("w2r", (128, 8), FP32, kind="ExternalInput").ap()
    conf = nc.dram_tensor("conf", (1, P), FP32, kind="ExternalInput").ap()
    b2 = nc.dram_tensor("b2", (1, 1), FP32, kind="ExternalInput").ap()
    out_s = nc.dram_tensor("out_s", (B, ROWS, T), FP32, kind="ExternalOutput").ap()

    with TileContext(nc) as tc:
        with tc.tile_pool(name="const", bufs=1) as const_pool, \
             tc.tile_pool(name="mlp", bufs=1) as mlp_pool, \
             tc.tile_pool(name="rot", bufs=2) as rot_pool, \
             tc.tile_pool(name="small", bufs=1) as small_pool, \
             tc.tile_pool(name="psA", bufs=1, space="PSUM") as psA, \
             tc.tile_pool(name="psB", bufs=1, space="PSUM") as psB, \
             tc.tile_pool(name="biasp", bufs=BIAS_BUFS) as bias_pool, \
             tc.tile_pool(name="accp", bufs=1) as acc_pool, \
             tc.tile_pool(name="attnp", bufs=B) as attn_pool:

            # ---- small consts on the Sync queue ----
            trigt = const_pool.tile([128, 8 * P], FP32, tag="trigp", name="trigt")
            nc.sync.dma_start(out=trigt, in_=trigp)
            ctxt = const_pool.tile([128, 8 * B], FP32, tag="ctxp", name="ctxt")
            nc.sync.dma_start(out=ctxt, in_=ctxp)
            b1t = const_pool.tile([1, C], FP32, tag="b1row", name="b1t")
            nc.sync.dma_start(out=b1t, in_=b1row)
            w2t = const_pool.tile([128, 8], FP32, tag="w2r", name="w2t")
            nc.sync.dma_start(out=w2t, in_=w2r)
            conft = const_pool.tile([1, P], FP32, tag="conf", name="conft")
            nc.sync.dma_start(out=conft, in_=conf)
            b2t = const_pool.tile([1, 1], FP32, tag="b2", name="b2t")
            nc.sync.dma_start(out=b2t, in_=b2)
            # W1 in chunk-aligned 2MB parts so chunk-0 matmuls start early
            w1hit = const_pool.tile([128, 8192], FP32, tag="w1hi", name="w1hit")
            w1lot = const_pool.tile([128, 8192], FP32, tag="w1lo", name="w1lot")
            for ch in range(2):
                csl = slice(ch * 4096, (ch + 1) * 4096)
                nc.sync.dma_start(out=w1hit[:, csl], in_=w1hi[:, csl])
                nc.sync.dma_start(out=w1lot[:, csl], in_=w1lo[:, csl])

            ones = const_pool.tile([1, 128], FP32, tag="ones", name="ones")
            nc.vector.memset(ones, 1.0)
            ident = const_pool.tile([128, 128], FP32, tag="ident", name="ident")
            from concourse.masks import make_identity
            make_identity(nc, ident)

            # ---- phase A: trg_hT [p, c], ctx_hT [b, c] (W1 moving) ----
            ps_trg = psA.tile([P, C], FP32, tag="trg", name="ps_trg")
            ps_ctx = psA.tile([B, C], FP32, tag="ctx", name="ps_ctx")
            trg_hsb = mlp_pool.tile([P, C], FP32, tag="trg_hsb", name="trg_hsb")
            ctx_hsb = mlp_pool.tile([B, C], FP32, tag="ctx_hsb", name="ctx_hsb")
            log_a = psB.tile([1, 512], FP32, tag="log_a", name="log_a")
            log_b = psB.tile([1, 288], FP32, tag="log_b", name="log_b")

            def phase_a(ch):
                osl = slice(ch * 512, (ch + 1) * 512)
                for kt in range(8):
                    wsl = slice(ch * 4096 + kt * 512, ch * 4096 + (kt + 1) * 512)
                    nc.tensor.matmul(ps_trg[:, osl],
                                     lhsT=trigt[:, kt * P:(kt + 1) * P],
                                     rhs=w1hit[:, wsl],
                                     start=(kt == 0), stop=(kt == 7))
                for kt in range(8):
                    wsl = slice(ch * 4096 + kt * 512, ch * 4096 + (kt + 1) * 512)
                    nc.tensor.matmul(ps_ctx[:, osl],
                                     lhsT=ctxt[:, kt * B:(kt + 1) * B],
                                     rhs=w1lot[:, wsl],
                                     start=(kt == 0), stop=False)
                # fold b1 in: ctx_hT[b, c] += 1 * b1[c]
                nc.tensor.matmul(ps_ctx[:, osl], lhsT=ones[0:1, 0:B],
                                 rhs=b1t[0:1, osl], start=False, stop=True)
                nc.scalar.activation(out=trg_hsb[:, osl], in_=ps_trg[:, osl],
                                     func=AF.Copy)
                nc.scalar.activation(out=ctx_hsb[:, osl], in_=ps_ctx[:, osl],
                                     func=AF.Copy)

            def phase_b(ct):
                csl = slice(ct * 128, (ct + 1) * 128)
                t2 = psB.tile([128, P], FP32, tag="t2", name=f"t2_{ct}")
                nc.tensor.transpose(t2, trg_hsb[:, csl], ident[0:P, 0:P])
                bv = psB.tile([128, B], FP32, tag="bv", name=f"bv_{ct}")
                nc.tensor.transpose(bv, ctx_hsb[:, csl], ident[0:B, 0:B])
                bvs = rot_pool.tile([128, B], FP32, tag="bvs", name=f"bvs_{ct}")
                nc.vector.tensor_copy(out=bvs, in_=bv)
                hT = rot_pool.tile([128, B * P], FP32, tag="hT", name=f"hT_{ct}")
                for b in range(4):
                    nc.scalar.activation(out=hT[:, b * P:(b + 1) * P], in_=t2,
                                         func=AF.Relu, bias=bvs[:, b:b + 1])
                for b in range(4, B):
                    nc.vector.tensor_scalar(out=hT[:, b * P:(b + 1) * P],
                                            in0=t2, scalar1=bvs[:, b:b + 1],
                                            scalar2=0.0, op0=ALU.add,
                                            op1=ALU.max)
                nc.tensor.matmul(log_a, lhsT=w2t[:, ct:ct + 1],
                                 rhs=hT[:, 0:512],
                                 start=(ct == 0), stop=(ct == 7))
                nc.tensor.matmul(log_b, lhsT=w2t[:, ct:ct + 1],
                                 rhs=hT[:, 512:800],
                                 start=(ct == 0), stop=(ct == 7))

            phase_a(0)
            for ct in range(4):
                phase_b(ct)
            phase_a(1)
            for ct in range(4, 8):
                phase_b(ct)

            # ---- scores -> weights (tiny, [1, *] on one partition) ----
            sig = small_pool.tile([1, B * P], FP32, tag="sig", name="sig")
            nc.scalar.activation(out=sig[:, 0:512], in_=log_a,
                                 func=AF.Sigmoid, bias=b2t[:, 0:1])
            nc.scalar.activation(out=sig[:, 512:800], in_=log_b,
                                 func=AF.Sigmoid, bias=b2t[:, 0:1])
            ssum = small_pool.tile([1, P], FP32, tag="ssum", name="ssum")
            nc.vector.tensor_add(out=ssum, in0=sig[:, 0:P], in1=sig[:, P:2 * P])
            for b in range(2, B):
                nc.vector.tensor_add(out=ssum, in0=ssum,
                                     in1=sig[:, b * P:(b + 1) * P])
            scores = small_pool.tile([1, P], FP32, tag="scores", name="scores")
            nc.vector.tensor_scalar_mul(out=scores, in0=ssum, scalar1=1.0 / B)
            mask = small_pool.tile([1, P], FP32, tag="mask", name="mask")
            nc.vector.tensor_scalar(out=mask, in0=scores, scalar1=SIM_THRESHOLD,
                                    scalar2=None, op0=ALU.is_gt)
            sc_conf = small_pool.tile([1, P], FP32, tag="sc_conf", name="sc_conf")
            nc.vector.tensor_mul(out=sc_conf, in0=scores, in1=conft)
            # w = (scores * conf * LAMBDA) * mask   (LAMBDA folded in here)
            w_vec = small_pool.tile([1, P], FP32, tag="w_vec", name="w_vec")
            nc.vector.scalar_tensor_tensor(out=w_vec, in0=sc_conf, scalar=LAMBDA,
                                           in1=mask, op0=ALU.mult, op1=ALU.mult)
            # broadcast w to all 128 partitions via rank-1 matmul
            wbc = psB.tile([128, P], FP32, tag="t2", name="wbc")
            nc.tensor.matmul(wbc, lhsT=ones, rhs=w_vec, start=True, stop=True)
            wsb = small_pool.tile([128, P], FP32, tag="wsb", name="wsb")
            nc.scalar.activation(out=wsb, in_=wbc, func=AF.Copy)

            # attn prefetch from the Scalar queue (issues after MLP ACT work,
            # keeping early HBM bandwidth for W1 + bias)
            attns = []
            for b in range(B):
                atile = attn_pool.tile([128, T], FP32, tag="attn", name=f"a{b}")
                nc.scalar.dma_start(out=atile, in_=attn_s[b])
                attns.append(atile)

            # ---- memory-bound phase: acc_c = sum_p w[p] * bias[p] (4 chains)
            accs = [acc_pool.tile([128, T], FP32, tag=f"acc{c}", name=f"acc{c}")
                    for c in range(4)]
            for p in range(P):
                btile = bias_pool.tile([128, T], FP32, tag="bias", name=f"b{p}")
                nc.gpsimd.dma_start(out=btile, in_=bias_s[p])
                ci = p // CHAIN
                if p % CHAIN == 0:
                    nc.vector.tensor_scalar_mul(out=accs[ci], in0=btile,
                                                scalar1=wsb[:, p:p + 1])
                else:
                    nc.vector.scalar_tensor_tensor(out=accs[ci], in0=btile,
                                                   scalar=wsb[:, p:p + 1],
                                                   in1=accs[ci], op0=ALU.mult,
                                                   op1=ALU.add)
                if p == 2 * CHAIN - 1:
                    # chains 0,1 done; merge in Vector's DMA-slack window
                    nc.vector.tensor_add(out=accs[0], in0=accs[0], in1=accs[1])

            # ---- tail: merge chains, add attn, store (split engines) ----
            nc.gpsimd.tensor_add(out=accs[2], in0=accs[2], in1=accs[3])
            nc.vector.tensor_add(out=accs[0], in0=accs[0], in1=accs[2])
            for b in range(B):
                eng = nc.vector if b < 4 else nc.gpsimd
                eng.tensor_add(out=attns[b], in0=attns[b], in1=accs[0])
                q = nc.sync if b % 2 == 0 else nc.scalar
                q.dma_start(out=out_s[b], in_=attns[b])

    # TRN2 matmul supports only one embedded semaphore wait; split the
    # extras onto InstEventSemaphore instructions (same pass Bacc runs).
    bass_rust.generate_event_semaphores(nc)
    return nc


def _get_nc() -> bass.Bass:
    if "nc" not in _NC_CACHE:
        _NC_CACHE["nc"] = _build_nc()
    return _NC_CACHE["nc"]


def _prep_in_maps(attention_scores, context, triggers, biases, confidences,
                  W1, b1, W2, b2):
    f32 = np.float32
    W1 = np.asarray(W1, dtype=f32)
    # [r, ch*4096 + kt*512 + c'] = W1half[kt*128 + r, ch*512 + c']
    w1hi_h = np.ascontiguousarray(
        W1[C:].reshape(8, 128, 2, 512).transpose(1, 2, 0, 3).reshape(128, 8192))
    w1lo_h = np.ascontiguousarray(
        W1[:C].reshape(8, 128, 2, 512).transpose(1, 2, 0, 3).reshape(128, 8192))
    trigp_h = np.ascontiguousarray(
        np.asarray(triggers, dtype=f32).T.reshape(8, 128, P)
        .transpose(1, 0, 2).reshape(128, 8 * P))
    ctxp_h = np.ascontiguousarray(
        np.asarray(context, dtype=f32).T.reshape(8, 128, B)
        .transpose(1, 0, 2).reshape(128, 8 * B))
    b1row_h = np.ascontiguousarray(np.asarray(b1, dtype=f32).reshape(1, C))
    w2r_h = np.ascontiguousarray(np.asarray(W2, dtype=f32).reshape(8, 128).T)
    conf_h = np.ascontiguousarray(np.asarray(confidences, dtype=f32).reshape(1, P))
    b2_h = np.ascontiguousarray(np.asarray(b2, dtype=f32).reshape(1, 1))
    attention_scores = np.asarray(attention_scores, dtype=f32)
    biases = np.asarray(biases, dtype=f32)
    in_maps = []
    for r in range(NCORES):
        rows = slice(r * ROWS, (r + 1) * ROWS)
        in_maps.append({
            "bias_s": np.ascontiguousarray(biases[:, rows, :]),
            "attn_s": np.ascontiguousarray(attention_scores[:, rows, :]),
            "w1hi": w1hi_h,
            "w1lo": w1lo_h,
            "trigp": trigp_h,
            "ctxp": ctxp_h,
            "b1row": b1row_h,
            "w2r": w2r_h,
            "conf": conf_h,
            "b2": b2_h,
        })
    return in_maps


def run(trace=False, **inputs):
    nc = _get_nc()
    in_maps = _prep_in_maps(**inputs)
    res = run_bass_kernel_spmd(nc, in_maps, core_ids=list(range(NCORES)),
                               trace=trace)
    out = np.concatenate([np.asarray(res.results[r]["out_s"])
                          for r in range(NCORES)], axis=1)
    return out.astype(np.float32), res


def kernel(**inputs) -> np.ndarray:
    out, _ = run(trace=False, **inputs)
    return out
